# revision 2
# baseline (speedup 1.0000x reference)
"""AffinityContrastiveLoss on 8 Trainium2 NeuronCores.

Sharding: mol axis across cores (2048 mols/core, all 2048 prots).
Per-core kernel computes, over its [2048 prot x 2048 mol] sim block:
  - row-wise sum(exp(sim)) partials        (p2m log-softmax denominator)
  - col-wise sum(exp(sim)) (full)          (m2p log-softmax denominator)
  - row-wise sum(l*pn) and sum(l*pn*r)     (affinity-weighted InfoNCE terms)
  - row-wise sum(relu(r))                  (negative push-down)
  - block-diagonal exp(sim) band           (positive pairs, via DRAM scratch
                                            + diagonal access-pattern DMA)
Host combines partials into the 5 scalar losses.
"""
import sys

for _p in ("/opt/trn_rl_repo", "/root/.axon_site/_ro/trn_rl_repo"):
    if _p not in sys.path:
        sys.path.insert(0, _p)

import numpy as np
import ml_dtypes
from contextlib import ExitStack

import concourse.bass as bass
import concourse.bacc as bacc
import concourse.tile as tile
import concourse.mybir as mybir
from concourse.bass_utils import run_bass_kernel_spmd

N_CORES = 8
N_PROTS = 2048
N_MOLS = 16384
DIM = 768
P = 8                       # mols per prot
MARGIN = 0.5
MPC = N_MOLS // N_CORES     # mols per core = 2048
PB = N_PROTS // 128         # prot blocks = 16
KC = DIM // 128             # contraction chunks = 6
NT = MPC // 512             # mol tiles per core = 4
BF16 = mybir.dt.bfloat16
F32 = mybir.dt.float32
F32R = mybir.dt.float32r

_cached = {}


def build_nc(scale: float):
    nc = bacc.Bacc("TRN2", target_bir_lowering=False, debug=False,
                   num_devices=N_CORES)
    protT = nc.dram_tensor("protT", [DIM, N_PROTS], F32, kind="ExternalInput")
    molT = nc.dram_tensor("molT", [DIM, MPC], F32, kind="ExternalInput")
    lab = nc.dram_tensor("lab", [N_PROTS, MPC], BF16, kind="ExternalInput")
    pic = nc.dram_tensor("pic", [N_PROTS, MPC], BF16, kind="ExternalInput")

    scratch = nc.dram_tensor("scratch", [PB, 128, 1024], BF16, kind="Internal")

    o_sexp = nc.dram_tensor("o_sexp", [128, PB * NT], F32, kind="ExternalOutput")
    o_u = nc.dram_tensor("o_u", [128, PB * NT], F32, kind="ExternalOutput")
    o_v = nc.dram_tensor("o_v", [128, PB * NT], F32, kind="ExternalOutput")
    o_relu = nc.dram_tensor("o_relu", [128, PB * NT], F32, kind="ExternalOutput")
    o_csum = nc.dram_tensor("o_csum", [1, MPC], F32, kind="ExternalOutput")
    o_band = nc.dram_tensor("o_band", [N_PROTS, P], BF16, kind="ExternalOutput")

    with tile.TileContext(nc) as tc, ExitStack() as ctx:
        const = ctx.enter_context(tc.tile_pool(name="const", bufs=1))
        emb = ctx.enter_context(tc.tile_pool(name="emb", bufs=1))
        slots = ctx.enter_context(tc.tile_pool(name="slots", bufs=1))
        lp_pool = ctx.enter_context(tc.tile_pool(name="lp", bufs=3))
        work = ctx.enter_context(tc.tile_pool(name="work", bufs=3))
        ps = ctx.enter_context(tc.tile_pool(name="ps", bufs=2, space="PSUM"))
        csps = ctx.enter_context(tc.tile_pool(name="csps", bufs=1, space="PSUM"))

        ones = const.tile([128, 1], BF16, tag="ones")
        nc.vector.memset(ones[:], 1.0)

        # resident transposed embeddings (float32r for full-rate fp32 matmul)
        ptT = emb.tile([128, KC, N_PROTS], F32R, tag="ptT")
        nc.sync.dma_start(
            ptT[:], protT.ap().rearrange("(c p) m -> p c m", p=128).bitcast(F32R))
        mtT = emb.tile([128, KC, MPC], F32R, tag="mtT")
        nc.sync.dma_start(
            mtT[:], molT.ap().rearrange("(c p) m -> p c m", p=128).bitcast(F32R))

        # per-quantity accumulation slot strips [128, pb*4+nt]
        sexp_s = slots.tile([128, PB * NT], F32, tag="sexp_s")
        u_s = slots.tile([128, PB * NT], F32, tag="u_s")
        v_s = slots.tile([128, PB * NT], F32, tag="v_s")
        relu_s = slots.tile([128, PB * NT], F32, tag="relu_s")

        cs_tiles = [csps.tile([1, 512], F32, tag=f"cs{nt}", name=f"cs{nt}")
                    for nt in range(NT)]

        for pb in range(PB):
            lab_t = lp_pool.tile([128, MPC], BF16, tag="lab_t")
            nc.sync.dma_start(lab_t[:], lab.ap()[pb * 128:(pb + 1) * 128, :])
            pic_t = lp_pool.tile([128, MPC], BF16, tag="pic_t")
            nc.sync.dma_start(pic_t[:], pic.ap()[pb * 128:(pb + 1) * 128, :])
            # pn = 0.125*pic - 0.25  (pic50 -> normalized affinity weight)
            pn_t = lp_pool.tile([128, MPC], BF16, tag="pn_t")
            nc.gpsimd.tensor_scalar(pn_t[:], pic_t[:], 0.125, -0.25,
                                    mybir.AluOpType.mult, mybir.AluOpType.add)

            for nt in range(NT):
                si = pb * NT + nt
                r_ps = ps.tile([128, 512], F32, tag="r_ps")
                for c in range(KC):
                    nc.tensor.matmul(r_ps[:],
                                     ptT[:, c, pb * 128:(pb + 1) * 128],
                                     mtT[:, c, nt * 512:(nt + 1) * 512],
                                     start=(c == 0), stop=(c == KC - 1))

                # exp(scale*r) -> bf16, with per-row sum accumulated
                exp_t = work.tile([128, 512], BF16, tag="exp_t")
                nc.scalar.activation(exp_t[:], r_ps[:],
                                     mybir.ActivationFunctionType.Exp,
                                     scale=scale,
                                     accum_out=sexp_s[:, si:si + 1])
                # relu(r) with per-row sum (negative push-down term)
                junk_r = work.tile([128, 512], BF16, tag="junk_r")
                nc.scalar.activation(junk_r[:], r_ps[:],
                                     mybir.ActivationFunctionType.Relu,
                                     accum_out=relu_s[:, si:si + 1])

                # lpn = lab * pn, rowsum -> u
                lpn_t = work.tile([128, 512], BF16, tag="lpn_t")
                nc.vector.scalar_tensor_tensor(
                    lpn_t[:], lab_t[:, nt * 512:(nt + 1) * 512], 0.0,
                    pn_t[:, nt * 512:(nt + 1) * 512],
                    mybir.AluOpType.bypass, mybir.AluOpType.mult,
                    accum_out=u_s[:, si:si + 1])
                # lpn * r, rowsum -> v
                junk_v = work.tile([128, 512], BF16, tag="junk_v")
                nc.vector.scalar_tensor_tensor(
                    junk_v[:], lpn_t[:], 0.0, r_ps[:],
                    mybir.AluOpType.bypass, mybir.AluOpType.mult,
                    accum_out=v_s[:, si:si + 1])

                # column sums of exp accumulate over prot blocks (PE ones-mm)
                nc.tensor.matmul(cs_tiles[nt][:], ones[:], exp_t[:],
                                 start=(pb == 0), stop=(pb == PB - 1))

                # block-diagonal band -> scratch (positives live here)
                if (nt // 2) == (pb % 2):
                    nc.sync.dma_start(
                        scratch.ap()[pb, :, (nt % 2) * 512:(nt % 2) * 512 + 512],
                        exp_t[:])

        # emit outputs
        nc.sync.dma_start(o_sexp.ap(), sexp_s[:])
        nc.sync.dma_start(o_u.ap(), u_s[:])
        nc.sync.dma_start(o_v.ap(), v_s[:])
        nc.sync.dma_start(o_relu.ap(), relu_s[:])

        cs_sb = const.tile([1, MPC], F32, tag="cs_sb")
        for nt in range(NT):
            nc.vector.tensor_copy(cs_sb[:, nt * 512:(nt + 1) * 512],
                                  cs_tiles[nt][:])
        nc.sync.dma_start(o_csum.ap(), cs_sb[:])

        for s in range(PB):
            src = bass.AP(scratch, s * 128 * 1024, [[1032, 128], [1, 8]])
            nc.sync.dma_start(o_band.ap()[s * 128:(s + 1) * 128, :], src)

    nc.compile()
    return nc


def _prepare_in_maps(prot_emb, mol_emb, labels, pic50_matrix):
    bf = ml_dtypes.bfloat16
    protT = np.ascontiguousarray(prot_emb.T)
    in_maps = []
    for c in range(N_CORES):
        cols = slice(c * MPC, (c + 1) * MPC)
        in_maps.append({
            "protT": protT,
            "molT": np.ascontiguousarray(mol_emb[cols].T),
            "lab": np.ascontiguousarray(labels[:, cols]).astype(bf),
            "pic": np.ascontiguousarray(pic50_matrix[:, cols]).astype(bf),
        })
    return in_maps


def _combine(results, pic50_matrix, s):
    f8 = np.float64
    sexp = np.zeros(N_PROTS, f8)
    u = np.zeros(N_PROTS, f8)
    vraw = np.zeros(N_PROTS, f8)
    relu_tot = f8(0.0)
    lse_col = np.zeros(N_MOLS, f8)
    band = np.zeros((N_PROTS, P), f8)
    for c, r in enumerate(results):
        # slot strips [128, pb*4+nt]: row pb*128+p <- strip[p, pb*4 : pb*4+4]
        def rows(a):
            # [128, 16*4] -> [16, 128] rowsums -> flatten to [2048]
            return a.astype(f8).reshape(128, PB, NT).sum(2).T.reshape(-1)
        sexp += rows(r["o_sexp"])
        u += rows(r["o_u"])
        vraw += rows(r["o_v"])
        relu_tot += r["o_relu"].astype(f8).sum()
        lse_col[c * MPC:(c + 1) * MPC] = np.log(r["o_csum"][0].astype(f8))
        own = slice(c * N_PROTS // N_CORES, (c + 1) * N_PROTS // N_CORES)
        band[own] = np.log(r["o_band"][own].astype(f8))

    lse_row = np.log(sexp)
    v = s * vraw
    loss_p2m = -np.mean((v - u * lse_row) / (u + 1e-8))

    n = band.reshape(-1)  # n[8i+a] = sim[i, 8i+a]
    loss_m2p = -np.mean(n - lse_col)

    # pairwise margin ranking among the P positives of each prot
    idx = np.arange(N_PROTS)[:, None] * P + np.arange(P)[None, :]
    pos_pic = pic50_matrix.astype(f8)[np.arange(N_PROTS)[:, None], idx]
    dp = pos_pic[:, :, None] - pos_pic[:, None, :]
    ds = band[:, :, None] - band[:, None, :]
    pair = np.where(dp > 0, np.maximum(MARGIN - ds, 0.0),
                    np.where(dp < 0, np.maximum(MARGIN + ds, 0.0), 0.0))
    upper = np.triu(np.ones((P, P), dtype=bool), k=1)
    n_pairs = N_PROTS * (P * (P - 1) // 2)
    ranking_loss = np.sum(np.where(upper[None], pair, 0.0)) / n_pairs

    # negative push-down: sum(relu(sim)) minus the positives' contribution
    neg_loss = (s * relu_tot - np.maximum(n, 0.0).sum()) / (N_PROTS * N_MOLS)

    total = loss_p2m + loss_m2p + 0.5 * ranking_loss + 0.1 * neg_loss
    return tuple(np.float32(x) for x in
                 (total, loss_p2m, loss_m2p, ranking_loss, neg_loss))


def kernel(prot_emb, mol_emb, labels, pic50_matrix, logit_scale):
    prot_emb = np.asarray(prot_emb, dtype=np.float32)
    mol_emb = np.asarray(mol_emb, dtype=np.float32)
    labels = np.asarray(labels, dtype=np.float32)
    pic50_matrix = np.asarray(pic50_matrix, dtype=np.float32)
    s = float(np.asarray(logit_scale))

    if "nc" not in _cached or _cached.get("scale") != s:
        _cached["nc"] = build_nc(s)
        _cached["scale"] = s
    nc = _cached["nc"]

    in_maps = _prepare_in_maps(prot_emb, mol_emb, labels, pic50_matrix)
    res = run_bass_kernel_spmd(nc, in_maps, core_ids=list(range(N_CORES)))
    return _combine(res.results, pic50_matrix, s)


if __name__ == "__main__":
    # quick self-test with random data through the simulator-free HW path
    rng = np.random.default_rng(0)
    pe = rng.standard_normal((N_PROTS, DIM)).astype(np.float32)
    pe /= np.linalg.norm(pe, axis=1, keepdims=True)
    me = rng.standard_normal((N_MOLS, DIM)).astype(np.float32)
    me /= np.linalg.norm(me, axis=1, keepdims=True)
    rows = np.repeat(np.arange(N_PROTS), P)
    lab = np.zeros((N_PROTS, N_MOLS), np.float32)
    lab[rows, np.arange(N_MOLS)] = 1.0
    pic = (2.0 + 8.0 * rng.random((N_PROTS, N_MOLS))).astype(np.float32)
    out = kernel(pe, me, lab, pic, np.float32(1.0 / 0.07))
    print("kernel out:", out)


# revision 10
# speedup vs baseline: 1.0101x; 1.0101x over previous
"""AffinityContrastiveLoss on 8 Trainium2 NeuronCores.

Sharding: mol axis across cores (2048 mols/core, all 2048 prots).
Per-core kernel computes, over its [2048 prot x 2048 mol] sim block:
  - row-wise sum(exp(sim)) partials        (p2m log-softmax denominator)
  - col-wise sum(exp(sim)) (full)          (m2p log-softmax denominator)
  - row-wise sum(l*pn) and sum(l*pn*r)     (affinity-weighted InfoNCE terms)
  - row-wise sum(relu(r))                  (negative push-down)
  - block-diagonal exp(sim) band           (positive pairs, via DRAM scratch
                                            + diagonal access-pattern DMA)
Host combines partials into the 5 scalar losses.
"""
import sys

for _p in ("/opt/trn_rl_repo", "/root/.axon_site/_ro/trn_rl_repo"):
    if _p not in sys.path:
        sys.path.insert(0, _p)

import numpy as np
import ml_dtypes
from contextlib import ExitStack

import concourse.bass as bass
import concourse.bacc as bacc
import concourse.tile as tile
import concourse.mybir as mybir
from concourse.bass_utils import run_bass_kernel_spmd

N_CORES = 8
N_PROTS = 2048
N_MOLS = 16384
DIM = 768
P = 8                       # mols per prot
MARGIN = 0.5
MPC = N_MOLS // N_CORES     # mols per core = 2048
PB = N_PROTS // 128         # prot blocks = 16
KC = DIM // 128             # contraction chunks = 6
TW = 1024                   # tile width (mol cols per compute tile)
NT = MPC // TW              # mol tiles per core = 2
BF16 = mybir.dt.bfloat16
F32 = mybir.dt.float32
F32R = mybir.dt.float32r

_cached = {}


def build_nc(scale: float):
    nc = bacc.Bacc("TRN2", target_bir_lowering=False, debug=False,
                   num_devices=N_CORES)
    protT = nc.dram_tensor("protT", [DIM, N_PROTS], F32, kind="ExternalInput")
    molT = nc.dram_tensor("molT", [DIM, MPC], F32, kind="ExternalInput")
    lab = nc.dram_tensor("lab", [N_PROTS, MPC], BF16, kind="ExternalInput")
    pic = nc.dram_tensor("pic", [N_PROTS, MPC], BF16, kind="ExternalInput")

    scratch = nc.dram_tensor("scratch", [PB, 128, TW], BF16, kind="Internal")

    o_sexp = nc.dram_tensor("o_sexp", [128, PB * NT], F32, kind="ExternalOutput")
    o_u = nc.dram_tensor("o_u", [128, PB * NT], F32, kind="ExternalOutput")
    o_v = nc.dram_tensor("o_v", [128, PB * NT], F32, kind="ExternalOutput")
    o_relu = nc.dram_tensor("o_relu", [128, PB * NT], F32, kind="ExternalOutput")
    o_csum = nc.dram_tensor("o_csum", [1, MPC], F32, kind="ExternalOutput")
    o_band = nc.dram_tensor("o_band", [N_PROTS, P], BF16, kind="ExternalOutput")

    with tile.TileContext(nc) as tc, ExitStack() as ctx:
        const = ctx.enter_context(tc.tile_pool(name="const", bufs=1))
        emb = ctx.enter_context(tc.tile_pool(name="emb", bufs=1))
        slots = ctx.enter_context(tc.tile_pool(name="slots", bufs=1))
        lp_pool = ctx.enter_context(tc.tile_pool(name="lp", bufs=3))
        work = ctx.enter_context(tc.tile_pool(name="work", bufs=3))
        ps = ctx.enter_context(tc.tile_pool(name="ps", bufs=3, space="PSUM"))
        csps = ctx.enter_context(tc.tile_pool(name="csps", bufs=1, space="PSUM"))

        ones = const.tile([128, 1], BF16, tag="ones")
        nc.vector.memset(ones[:], 1.0)

        # resident transposed embeddings (float32r for full-rate fp32 matmul),
        # loaded in k-chunk pieces so the first matmuls can start early
        ptT = emb.tile([128, KC, N_PROTS], F32R, tag="ptT")
        mtT = emb.tile([128, KC, MPC], F32R, tag="mtT")
        pt_src = protT.ap().rearrange("(c p) m -> p c m", p=128).bitcast(F32R)
        mt_src = molT.ap().rearrange("(c p) m -> p c m", p=128).bitcast(F32R)
        h0 = slice(0, MPC // 2)
        h1 = slice(MPC // 2, MPC)
        for c in range(KC):
            nc.sync.dma_start(mtT[:, c, h0], mt_src[:, c, h0])
            nc.sync.dma_start(ptT[:, c, :], pt_src[:, c, :])
        for c in range(KC):
            nc.sync.dma_start(mtT[:, c, h1], mt_src[:, c, h1])

        # per-quantity accumulation slot strips [128, pb*NT+nt]
        sexp_s = slots.tile([128, PB * NT], F32, tag="sexp_s")
        u_s = slots.tile([128, PB * NT], F32, tag="u_s")
        v_s = slots.tile([128, PB * NT], F32, tag="v_s")
        relu_s = slots.tile([128, PB * NT], F32, tag="relu_s")

        # 4 column-sum accumulators packed into one PSUM bank at
        # partitions {0,32,64,96} (matmul output base partition must be
        # 32-aligned)
        cs_all = csps.tile([128, 512], F32, tag="cs_all")

        for pb in range(PB):
            lab_t = lp_pool.tile([128, MPC], BF16, tag="lab_t")
            nc.sync.dma_start(lab_t[:], lab.ap()[pb * 128:(pb + 1) * 128, :])
            pic_t = lp_pool.tile([128, MPC], BF16, tag="pic_t")
            nc.sync.dma_start(pic_t[:], pic.ap()[pb * 128:(pb + 1) * 128, :])
            # pn = 0.125*pic - 0.25  (pic50 -> normalized affinity weight)
            pn_t = lp_pool.tile([128, MPC], BF16, tag="pn_t")
            nc.gpsimd.tensor_scalar(pn_t[:], pic_t[:], 0.125, -0.25,
                                    mybir.AluOpType.mult, mybir.AluOpType.add)

            for nt in range(NT):
                si = pb * NT + nt
                r_ps = ps.tile([128, TW], F32, tag="r_ps")
                for h in range(TW // 512):
                    for c in range(KC):
                        nc.tensor.matmul(
                            r_ps[:, h * 512:(h + 1) * 512],
                            ptT[:, c, pb * 128:(pb + 1) * 128],
                            mtT[:, c, nt * TW + h * 512:nt * TW + (h + 1) * 512],
                            start=(c == 0), stop=(c == KC - 1))

                # exp(scale*r) -> bf16, with per-row sum accumulated
                exp_t = work.tile([128, TW], BF16, tag="exp_t")
                nc.scalar.activation(exp_t[:], r_ps[:],
                                     mybir.ActivationFunctionType.Exp,
                                     scale=scale,
                                     accum_out=sexp_s[:, si:si + 1])
                # relu(r) with per-row sum (negative push-down term)
                junk_r = work.tile([128, TW], BF16, tag="junk_r")
                nc.scalar.activation(junk_r[:], r_ps[:],
                                     mybir.ActivationFunctionType.Relu,
                                     accum_out=relu_s[:, si:si + 1])

                # lpn = lab * pn, rowsum -> u
                lpn_t = work.tile([128, TW], BF16, tag="lpn_t")
                nc.vector.scalar_tensor_tensor(
                    lpn_t[:], lab_t[:, nt * TW:(nt + 1) * TW], 0.0,
                    pn_t[:, nt * TW:(nt + 1) * TW],
                    mybir.AluOpType.bypass, mybir.AluOpType.mult,
                    accum_out=u_s[:, si:si + 1])
                # lpn * r, rowsum -> v
                junk_v = work.tile([128, TW], BF16, tag="junk_v")
                nc.vector.scalar_tensor_tensor(
                    junk_v[:], lpn_t[:], 0.0, r_ps[:],
                    mybir.AluOpType.bypass, mybir.AluOpType.mult,
                    accum_out=v_s[:, si:si + 1])

                # column sums of exp accumulate over prot blocks (PE ones-mm)
                for h in range(TW // 512):
                    g = nt * (TW // 512) + h
                    nc.tensor.matmul(cs_all[32 * g:32 * g + 1, :],
                                     ones[:], exp_t[:, h * 512:(h + 1) * 512],
                                     start=(pb == 0), stop=(pb == PB - 1),
                                     tile_position=(0, 32 * g))

                # block-diagonal band -> scratch (positives live here)
                if nt == (pb % 2):
                    nc.sync.dma_start(scratch.ap()[pb], exp_t[:])
                    # extract the 8 positives per row as soon as the slab lands
                    nc.sync.dma_start(
                        o_band.ap()[pb * 128:(pb + 1) * 128, :],
                        bass.AP(scratch, pb * 128 * TW, [[TW + P, 128], [1, P]]))

        # emit outputs
        nc.sync.dma_start(o_sexp.ap(), sexp_s[:])
        nc.sync.dma_start(o_u.ap(), u_s[:])
        nc.sync.dma_start(o_v.ap(), v_s[:])
        nc.sync.dma_start(o_relu.ap(), relu_s[:])

        cs_sb = const.tile([128, 512], F32, tag="cs_sb")
        nc.vector.tensor_copy(cs_sb[:], cs_all[:])
        # rows {0,32,64,96} of cs_sb are the 4 column-sum groups
        for g in range(4):
            nc.sync.dma_start(o_csum.ap()[:, g * 512:(g + 1) * 512],
                              cs_sb[32 * g:32 * g + 1, :])

    nc.compile()
    return nc


def _prepare_in_maps(prot_emb, mol_emb, labels, pic50_matrix):
    bf = ml_dtypes.bfloat16
    protT = np.ascontiguousarray(prot_emb.T)
    in_maps = []
    for c in range(N_CORES):
        cols = slice(c * MPC, (c + 1) * MPC)
        in_maps.append({
            "protT": protT,
            "molT": np.ascontiguousarray(mol_emb[cols].T),
            "lab": np.ascontiguousarray(labels[:, cols]).astype(bf),
            "pic": np.ascontiguousarray(pic50_matrix[:, cols]).astype(bf),
        })
    return in_maps


def _combine(results, pic50_matrix, s):
    f8 = np.float64
    sexp = np.zeros(N_PROTS, f8)
    u = np.zeros(N_PROTS, f8)
    vraw = np.zeros(N_PROTS, f8)
    relu_tot = f8(0.0)
    lse_col = np.zeros(N_MOLS, f8)
    band = np.zeros((N_PROTS, P), f8)
    for c, r in enumerate(results):
        # slot strips [128, pb*NT+nt]: row pb*128+p <- sum over nt
        def rows(a):
            return a.astype(f8).reshape(128, PB, NT).sum(2).T.reshape(-1)
        sexp += rows(r["o_sexp"])
        u += rows(r["o_u"])
        vraw += rows(r["o_v"])
        relu_tot += r["o_relu"].astype(f8).sum()
        lse_col[c * MPC:(c + 1) * MPC] = np.log(r["o_csum"][0].astype(f8))
        own = slice(c * N_PROTS // N_CORES, (c + 1) * N_PROTS // N_CORES)
        band[own] = np.log(r["o_band"][own].astype(f8))

    lse_row = np.log(sexp)
    v = s * vraw
    loss_p2m = -np.mean((v - u * lse_row) / (u + 1e-8))

    n = band.reshape(-1)  # n[8i+a] = sim[i, 8i+a]
    loss_m2p = -np.mean(n - lse_col)

    # pairwise margin ranking among the P positives of each prot
    idx = np.arange(N_PROTS)[:, None] * P + np.arange(P)[None, :]
    pos_pic = pic50_matrix.astype(f8)[np.arange(N_PROTS)[:, None], idx]
    dp = pos_pic[:, :, None] - pos_pic[:, None, :]
    ds = band[:, :, None] - band[:, None, :]
    pair = np.where(dp > 0, np.maximum(MARGIN - ds, 0.0),
                    np.where(dp < 0, np.maximum(MARGIN + ds, 0.0), 0.0))
    upper = np.triu(np.ones((P, P), dtype=bool), k=1)
    n_pairs = N_PROTS * (P * (P - 1) // 2)
    ranking_loss = np.sum(np.where(upper[None], pair, 0.0)) / n_pairs

    # negative push-down: sum(relu(sim)) minus the positives' contribution
    neg_loss = (s * relu_tot - np.maximum(n, 0.0).sum()) / (N_PROTS * N_MOLS)

    total = loss_p2m + loss_m2p + 0.5 * ranking_loss + 0.1 * neg_loss
    return tuple(np.float32(x) for x in
                 (total, loss_p2m, loss_m2p, ranking_loss, neg_loss))


def kernel(prot_emb, mol_emb, labels, pic50_matrix, logit_scale):
    prot_emb = np.asarray(prot_emb, dtype=np.float32)
    mol_emb = np.asarray(mol_emb, dtype=np.float32)
    labels = np.asarray(labels, dtype=np.float32)
    pic50_matrix = np.asarray(pic50_matrix, dtype=np.float32)
    s = float(np.asarray(logit_scale))

    if "nc" not in _cached or _cached.get("scale") != s:
        _cached["nc"] = build_nc(s)
        _cached["scale"] = s
    nc = _cached["nc"]

    in_maps = _prepare_in_maps(prot_emb, mol_emb, labels, pic50_matrix)
    res = run_bass_kernel_spmd(nc, in_maps, core_ids=list(range(N_CORES)))
    return _combine(res.results, pic50_matrix, s)


if __name__ == "__main__":
    rng = np.random.default_rng(0)
    pe = rng.standard_normal((N_PROTS, DIM)).astype(np.float32)
    pe /= np.linalg.norm(pe, axis=1, keepdims=True)
    me = rng.standard_normal((N_MOLS, DIM)).astype(np.float32)
    me /= np.linalg.norm(me, axis=1, keepdims=True)
    rows = np.repeat(np.arange(N_PROTS), P)
    lab = np.zeros((N_PROTS, N_MOLS), np.float32)
    lab[rows, np.arange(N_MOLS)] = 1.0
    pic = (2.0 + 8.0 * rng.random((N_PROTS, N_MOLS))).astype(np.float32)
    out = kernel(pe, me, lab, pic, np.float32(1.0 / 0.07))
    print("kernel out:", out)
